# revision 36
# baseline (speedup 1.0000x reference)
"""CFConv (SchNet continuous-filter convolution) on 8 TRN2 NeuronCores.

Reference computation:
    f    = x @ W_in                       # (20000, 128)
    f_j  = f[idx_j]                       # (640000, 128) gather
    wf   = w_ij * f_j                     # elementwise
    conv = segment_sum(wf, seg_i)         # (20000, 128), seg_i sorted
    out  = conv @ W_out + b_out

Distribution: seg_i is sorted, so atoms are split into 8 contiguous
ranges of 2560 (padded to 20480); each core gets the edges targeting its
atom range.  No collectives needed — each core owns its output rows.

Per-core device pipeline (v4):
  Phase A: f = x @ W_in computed locally (replicated), written to one
           internal HBM table in PARTITION-MAJOR row order
           (row = p*160 + j) so the table write uses 2KB descriptors.
  Phase B: edges processed in groups of 128 (one group = one matmul
           contraction), host-packed per 128-atom window and padded to a
           uniform k groups/window so the graph is identical on all
           cores.
    - w_ij group tiles DMA'd from HBM (host-reordered, bf16)
    - f_j rows fetched with gpsimd.dma_gather, FOUR calls per window,
      one on each SWDGE queue: each queue's descriptors drain through a
      fixed 4-of-16 subset of DMA engines, so a window must touch all 4
      queues to keep all 16 engines busy.
    - wf = w * f_j on VectorE
    - segment-sum via TensorE: psum[fm, atom_window] += wf_g^T @ S_g
      where S_g is the host-built 0/1 edge->atom one-hot (fp8 rhs)
    - out^T = W_out^T @ conv^T (TensorE), bias via ScalarE, written to
      HBM transposed (fo-major, 2KB descriptors); the host untransposes.

Atoms are host-relabeled (snake-deal by per-atom edge count) so every
window carries a near-equal edge count, minimizing the uniform padding;
the output is un-permuted on the host after the gather.

dynamic_dma_scratch_size is raised 16K->48K so each SWDGE queue's
descriptor ring holds ~6 gather calls instead of ~2; without this the
gather issue blocks in await_space and the queues starve between
windows.

Measured on 8 axon TRN2 cores: ~296 us HW exec (baseline 316 us),
rel err 4.6e-3 vs the f32 reference.  Known structure: the gather
drains at ~7.9 ns/idx per SWDGE queue (4 queues, each through ~4 of
the 16 DMA engines) which puts the phase-B floor at ~8.1 us/window;
ap_gather on GpSimd measured ~27 ns/idx (useless), prepare_only+
trigger_dma costs 1.4 us/trigger in IncSwdgeSem (net loss).
"""

import numpy as np
import ml_dtypes

import concourse.bacc as bacc
import concourse.bass as bass
import concourse.mybir as mybir
import concourse.tile as tile
from concourse.bass_utils import run_bass_kernel_spmd

BF16 = ml_dtypes.bfloat16
FP8 = ml_dtypes.float8_e4m3

N_ATOMS = 20000
N_EDGES = 640000
F = 128
N_CORES = 8
A_CORE = 2560                 # padded atoms per core
A_PAD = A_CORE * N_CORES      # 20480
CHUNK = 512                   # atoms per PSUM chunk (one bank)
WIN = 128                     # atoms per window (matmul N dim)
WIN_PER_CORE = A_CORE // WIN  # 20
N_WIN = A_PAD // WIN          # 160
JROWS = A_PAD // 128          # 160 j-rows of 128 atoms

TRACE = False                 # set True (with ntff shim) for profiling
_BUILD_CACHE: dict = {}


def _build(k: int):
    """Build the SPMD Bass graph for k groups (of 128 edges) per window."""
    if k in _BUILD_CACHE:
        return _BUILD_CACHE[k]

    G = WIN_PER_CORE * k          # groups per core
    E = G * 128                   # padded edges per core
    bf = mybir.dt.bfloat16
    f32 = mybir.dt.float32

    nc = bacc.Bacc("TRN2", target_bir_lowering=False, debug=False,
                   num_swdge_queues=4, num_devices=N_CORES,
                   dynamic_dma_scratch_size=49152)
    xT_e = nc.dram_tensor("xT", [128, A_PAD], bf, kind="ExternalInput")
    w_in_e = nc.dram_tensor("w_in", [128, 128], bf, kind="ExternalInput")
    w_out_e = nc.dram_tensor("w_out", [128, 128], bf, kind="ExternalInput")
    b_e = nc.dram_tensor("b_out", [128, 1], f32, kind="ExternalInput")
    w_ed_e = nc.dram_tensor("w_ed", [128, G, F], bf, kind="ExternalInput")
    rel_e = nc.dram_tensor("rel_ed", [128, G], bf, kind="ExternalInput")
    iota_e = nc.dram_tensor("iota", [128, 128], bf, kind="ExternalInput")
    idx_e = nc.dram_tensor("idxw", [128, E // 16], mybir.dt.int16,
                           kind="ExternalInput")
    # out^T (fo-major); host untransposes.
    out_e = nc.dram_tensor("out", [128, A_CORE], f32, kind="ExternalOutput")

    with tile.TileContext(nc) as tc:
        with (
            tc.tile_pool(name="dram", bufs=1, space="DRAM") as dpool,
            tc.tile_pool(name="const", bufs=1) as cpool,
        ):
            # f table, partition-major: atom (p, j) -> row p*JROWS + j
            f_hbm = dpool.tile([128, JROWS, F], bf)

            w_in_t = cpool.tile([128, 128], bf)
            nc.sync.dma_start(w_in_t[:], w_in_e[:])
            w_out_t = cpool.tile([128, 128], bf)
            nc.sync.dma_start(w_out_t[:], w_out_e[:])
            b_t = cpool.tile([128, 1], f32)
            nc.sync.dma_start(b_t[:], b_e[:])
            iota_t = cpool.tile([128, 128], bf)
            nc.sync.dma_start(iota_t[:], iota_e[:])
            rel_t = cpool.tile([128, G], bf)
            nc.scalar.dma_start(rel_t[:], rel_e[:])
            idx_t = cpool.tile([128, E // 16], mybir.dt.int16)
            nc.scalar.dma_start(idx_t[:], idx_e[:])

            # ---------------- Phase A: f table ----------------
            QW = A_PAD // 4
            with (
                tc.tile_pool(name="pha", bufs=4) as apool,
                tc.tile_pool(name="psA", bufs=3, space="PSUM") as psA,
            ):
                f_sb = None
                # hoist all chunk loads: they sit first on the sync ring
                # (FIFO), so no chunk load waits behind an f-write's sem
                xqs = []
                for x4 in range(4):
                    xq_t = apool.tile([128, QW], bf, tag="xq")
                    nc.sync.dma_start(xq_t[:], xT_e[:, x4 * QW:(x4 + 1) * QW])
                    xqs.append(xq_t)
                for x4 in range(4):
                    xq_t = xqs[x4]
                    for t4q in range(QW // 512):
                        t4 = x4 * (QW // 512) + t4q
                        ps = psA.tile([128, 4, 128], f32)
                        for q in range(4):
                            tl = t4q * 4 + q
                            nc.tensor.matmul(
                                ps[:, q, :],
                                xq_t[:, tl * 128:(tl + 1) * 128],
                                w_in_t[:],
                                start=True, stop=True,
                            )
                        j = t4 % 2
                        if j == 0:
                            f_sb = apool.tile([128, 8, F], bf, tag="fsb")
                        # split PSUM->SBUF casts across Vector and Scalar
                        if t4 % 2 == 0:
                            nc.vector.tensor_copy(
                                f_sb[:, j * 4:(j + 1) * 4, :], ps[:])
                        else:
                            nc.scalar.copy(
                                f_sb[:, j * 4:(j + 1) * 4, :], ps[:])
                        if j == 1:
                            m = t4 // 2          # j-rows [8m, 8m+8)
                            nc.sync.dma_start(
                                f_hbm[:, 8 * m:8 * m + 8, :], f_sb[:])

            # ---------------- Phase B: edges ----------------
            with (
                tc.tile_pool(name="phb", bufs=3) as bpool,
                tc.tile_pool(name="fjp", bufs=6) as fjpool,
                tc.tile_pool(name="psC", bufs=2, space="PSUM") as pscp,
                tc.tile_pool(name="ps2", bufs=2, space="PSUM") as ps2p,
            ):
                psc = None
                for wk in range(WIN_PER_CORE):
                    ch = wk // 4
                    col = WIN * (wk % 4)

                    # Delay the first windows' streaming loads so the
                    # phase-A table chain gets the DMA bandwidth first.
                    delay = 0.024 if wk < 3 else 0
                    with tc.tile_wait_until(delay, enable=wk < 3):
                        w_t = bpool.tile([128, k, F], bf, tag="w")
                        nc.scalar.dma_start(
                            w_t[:], w_ed_e[:, wk * k:(wk + 1) * k, :])
                    # S one-hot generated on DVE from rel values:
                    # S[e, g, a] = (rel[e, g] == a)
                    s_t = bpool.tile([128, k, WIN], mybir.dt.float8e4,
                                     tag="s")
                    nc.vector.tensor_tensor(
                        s_t[:],
                        rel_t[:, wk * k:(wk + 1) * k]
                        .unsqueeze(-1).broadcast_to([128, k, WIN]),
                        iota_t[:].unsqueeze(1).broadcast_to([128, k, WIN]),
                        mybir.AluOpType.is_equal)
                    base8 = wk * k * 8
                    fj_t = fjpool.tile([128, k, F], bf, tag="fj")
                    # 4 gather calls per window, one per SWDGE queue
                    kq = k // 4
                    for piece in range(4):
                        g0, g1 = piece * kq, (piece + 1) * kq
                        if piece == 3:
                            g1 = k
                        nc.gpsimd.dma_gather(
                            fj_t[:, g0:g1, :],
                            f_hbm[:].rearrange("p j f -> (p j) f"),
                            idx_t[:, base8 + g0 * 8:base8 + g1 * 8],
                            num_idxs=(g1 - g0) * 128,
                            num_idxs_reg=(g1 - g0) * 128,
                            elem_size=F,
                            single_packet=False,
                            queue_num=(piece + wk) % 4,
                        )

                    wf_t = bpool.tile([128, k, F], bf, tag="wf")
                    nc.vector.tensor_tensor(
                        wf_t[:], w_t[:], fj_t[:], mybir.AluOpType.mult)

                    if wk % 4 == 0:
                        psc = pscp.tile([128, CHUNK], f32)
                    for g in range(k):
                        nc.tensor.matmul(
                            psc[:, col:col + WIN],
                            wf_t[:, g, :],
                            s_t[:, g, :],
                            start=(g == 0), stop=(g == k - 1),
                        )

                    if wk % 4 == 3:
                        convT = bpool.tile([128, CHUNK], bf, tag="convT")
                        nc.vector.tensor_copy(convT[:], psc[:])
                        ps2 = ps2p.tile([128, CHUNK], f32)
                        nc.tensor.matmul(ps2[:], w_out_t[:], convT[:],
                                         start=True, stop=True)
                        outT = bpool.tile([128, CHUNK], f32, tag="outT")
                        nc.scalar.activation(
                            outT[:], ps2[:],
                            mybir.ActivationFunctionType.Identity,
                            bias=b_t[:],
                        )
                        nc.sync.dma_start(
                            out_e[:, ch * CHUNK:(ch + 1) * CHUNK], outT[:])

    nc.compile()
    _BUILD_CACHE[k] = nc
    return nc


def _prep(x, w_ij, seg_i, idx_j, W_in, W_out, b_out):
    """Host-side sharding: reorder/pad edges, build S one-hots, wrap idxs."""
    x = np.asarray(x, dtype=np.float32)
    w_ij = np.asarray(w_ij, dtype=np.float32)
    seg = np.asarray(seg_i).astype(np.int64)
    idxj = np.asarray(idx_j).astype(np.int64)

    # Relabel atoms so every 128-atom window gets a near-equal edge count
    # (snake-deal atoms in decreasing edge-count order over the windows).
    cnt = np.bincount(seg, minlength=N_ATOMS)
    order = np.argsort(-cnt, kind="stable")
    i = np.arange(N_ATOMS)
    r, c = np.divmod(i, N_WIN)
    w = np.where(r % 2 == 0, c, N_WIN - 1 - c)
    perm = np.empty(N_ATOMS, np.int64)
    perm[order] = w * WIN + r
    seg = perm[seg]
    idxj = perm[idxj]
    o = np.argsort(seg, kind="stable")
    seg, idxj, w_ij = seg[o], idxj[o], w_ij[o]

    bounds = np.searchsorted(seg, np.arange(N_WIN + 1) * WIN)
    n_win = np.diff(bounds)
    k = max(1, int(np.ceil(n_win.max() / 128)))
    e_win = k * 128
    g_core = WIN_PER_CORE * k
    e_pad = g_core * 128

    # Gather-table row for atom a: (p, j) = (a % 128, a // 128);
    # partition-major row = p*JROWS + j.
    grow = ((idxj % 128) * JROWS + idxj // 128).astype(np.int16)

    # padded edge-id + gather-idx matrices
    eidx = np.zeros((N_WIN, e_win), np.int64)
    valid = np.zeros((N_WIN, e_win), bool)
    gidx = np.zeros((N_WIN, e_win), np.int16)
    for kw in range(N_WIN):
        b0, b1 = bounds[kw], bounds[kw + 1]
        n = b1 - b0
        eidx[kw, :n] = np.arange(b0, b1)
        valid[kw, :n] = True
        gidx[kw, :n] = grow[b0:b1]

    w_bf = w_ij.astype(BF16)

    xT = np.zeros((128, A_PAD), BF16)
    xT[:, perm] = np.ascontiguousarray(x.T).astype(BF16)
    shared = {
        "xT": xT,
        "w_in": np.asarray(W_in, np.float32).astype(BF16),
        "w_out": np.asarray(W_out, np.float32).astype(BF16),
        "b_out": np.asarray(b_out, np.float32).reshape(128, 1).copy(),
        "iota": np.tile(np.arange(128, dtype=np.float32).astype(BF16),
                        (128, 1)),
    }

    in_maps = []
    for c in range(N_CORES):
        sl = slice(c * WIN_PER_CORE, (c + 1) * WIN_PER_CORE)
        ei = eidx[sl].reshape(-1)
        va = valid[sl].reshape(-1)

        w_rows = np.zeros((e_pad, F), BF16)
        w_rows[va] = w_bf[ei[va]]
        w_ed = np.ascontiguousarray(
            w_rows.reshape(g_core, 128, F).transpose(1, 0, 2))

        wb = (np.arange(c * WIN_PER_CORE, (c + 1) * WIN_PER_CORE)
              * WIN).repeat(e_win)
        rel = np.where(va, seg[ei] - wb, 0)
        # rel value per (e-partition, group), bf16 (0..127 exact)
        rel_ed = np.ascontiguousarray(
            rel.reshape(g_core, 128).T.astype(np.float32)).astype(BF16)

        # wrapped idx layout: per window, contiguous [16, k*8] wraps
        gi = gidx[sl]                              # [20, e_win]
        blocks = [gi[wkk].reshape(-1, 16).T for wkk in range(WIN_PER_CORE)]
        idxw = np.ascontiguousarray(
            np.tile(np.concatenate(blocks, axis=1), (8, 1)))

        m = dict(shared)
        m["w_ed"] = w_ed
        m["rel_ed"] = rel_ed
        m["idxw"] = idxw
        in_maps.append(m)
    return k, in_maps, perm


def kernel(x, w_ij, seg_i, idx_j, seg_i_sum, W_in, W_out, b_out):
    k, in_maps, perm = _prep(x, w_ij, seg_i, idx_j, W_in, W_out, b_out)
    nc = _build(k)
    res = run_bass_kernel_spmd(nc, in_maps, core_ids=list(range(N_CORES)),
                               trace=TRACE)
    kernel.last_result = res
    # out^T per core: [128 fo, 2560 atoms] -> [2560, 128]
    out = np.concatenate(
        [np.asarray(res.results[c]["out"]).T for c in range(N_CORES)], axis=0)
    return np.ascontiguousarray(out[perm]).astype(np.float32)


# revision 37
# speedup vs baseline: 1.0416x; 1.0416x over previous
"""CFConv (SchNet continuous-filter convolution) on 8 TRN2 NeuronCores.

Reference computation:
    f    = x @ W_in                       # (20000, 128)
    f_j  = f[idx_j]                       # (640000, 128) gather
    wf   = w_ij * f_j                     # elementwise
    conv = segment_sum(wf, seg_i)         # (20000, 128), seg_i sorted
    out  = conv @ W_out + b_out

Distribution: seg_i is sorted, so atoms are split into 8 contiguous
ranges of 2560 (padded to 20480); each core gets the edges targeting its
atom range.  No collectives needed — each core owns its output rows.

Per-core device pipeline (v4):
  Phase A: f = x @ W_in computed locally (replicated), written to one
           internal HBM table in PARTITION-MAJOR row order
           (row = p*160 + j) so the table write uses 2KB descriptors.
  Phase B: edges processed in groups of 128 (one group = one matmul
           contraction), host-packed per 128-atom window and padded to a
           uniform k groups/window so the graph is identical on all
           cores.
    - w_ij group tiles DMA'd from HBM (host-reordered, bf16)
    - f_j rows fetched with gpsimd.dma_gather, FOUR calls per window,
      one on each SWDGE queue: each queue's descriptors drain through a
      fixed 4-of-16 subset of DMA engines, so a window must touch all 4
      queues to keep all 16 engines busy.
    - wf = w * f_j on VectorE
    - segment-sum via TensorE: psum[fm, atom_window] += wf_g^T @ S_g
      where S_g is the host-built 0/1 edge->atom one-hot (fp8 rhs)
    - out^T = W_out^T @ conv^T (TensorE), bias via ScalarE, written to
      HBM transposed (fo-major, 2KB descriptors); the host untransposes.

Atoms are host-relabeled (snake-deal by per-atom edge count) so every
window carries a near-equal edge count, minimizing the uniform padding;
the output is un-permuted on the host after the gather.

dynamic_dma_scratch_size is raised 16K->48K so each SWDGE queue's
descriptor ring holds ~6 gather calls instead of ~2; without this the
gather issue blocks in await_space and the queues starve between
windows.

Measured on 8 axon TRN2 cores: ~296 us HW exec (baseline 316 us),
rel err 4.6e-3 vs the f32 reference.  Known structure: the gather
drains at ~7.9 ns/idx per SWDGE queue (4 queues, each through ~4 of
the 16 DMA engines) which puts the phase-B floor at ~8.1 us/window;
ap_gather on GpSimd measured ~27 ns/idx (useless), prepare_only+
trigger_dma costs 1.4 us/trigger in IncSwdgeSem (net loss).
"""

import numpy as np
import ml_dtypes

import concourse.bacc as bacc
import concourse.bass as bass
import concourse.mybir as mybir
import concourse.tile as tile
from concourse.bass_utils import run_bass_kernel_spmd

BF16 = ml_dtypes.bfloat16
FP8 = ml_dtypes.float8_e4m3

N_ATOMS = 20000
N_EDGES = 640000
F = 128
N_CORES = 8
A_CORE = 2560                 # padded atoms per core
A_PAD = A_CORE * N_CORES      # 20480
CHUNK = 512                   # atoms per PSUM chunk (one bank)
WIN = 128                     # atoms per window (matmul N dim)
WIN_PER_CORE = A_CORE // WIN  # 20
N_WIN = A_PAD // WIN          # 160
JROWS = A_PAD // 128          # 160 j-rows of 128 atoms

TRACE = False                 # set True (with ntff shim) for profiling
_BUILD_CACHE: dict = {}


def _build(k: int):
    """Build the SPMD Bass graph for k groups (of 128 edges) per window."""
    if k in _BUILD_CACHE:
        return _BUILD_CACHE[k]

    G = WIN_PER_CORE * k          # groups per core
    E = G * 128                   # padded edges per core
    bf = mybir.dt.bfloat16
    f32 = mybir.dt.float32

    nc = bacc.Bacc("TRN2", target_bir_lowering=False, debug=False,
                   num_swdge_queues=4, num_devices=N_CORES,
                   dynamic_dma_scratch_size=49152)
    xT_e = nc.dram_tensor("xT", [128, A_PAD], bf, kind="ExternalInput")
    w_in_e = nc.dram_tensor("w_in", [128, 128], bf, kind="ExternalInput")
    w_out_e = nc.dram_tensor("w_out", [128, 128], bf, kind="ExternalInput")
    b_e = nc.dram_tensor("b_out", [128, 1], f32, kind="ExternalInput")
    w_ed_e = nc.dram_tensor("w_ed", [128, G, F], bf, kind="ExternalInput")
    rel_e = nc.dram_tensor("rel_ed", [128, G], bf, kind="ExternalInput")
    iota_e = nc.dram_tensor("iota", [128, 128], bf, kind="ExternalInput")
    idx_e = nc.dram_tensor("idxw", [128, E // 16], mybir.dt.int16,
                           kind="ExternalInput")
    # out^T (fo-major); host untransposes.
    out_e = nc.dram_tensor("out", [128, A_CORE], f32, kind="ExternalOutput")

    with tile.TileContext(nc) as tc:
        with (
            tc.tile_pool(name="dram", bufs=1, space="DRAM") as dpool,
            tc.tile_pool(name="const", bufs=1) as cpool,
        ):
            # f table, partition-major: atom (p, j) -> row p*JROWS + j
            f_hbm = dpool.tile([128, JROWS, F], bf)

            w_in_t = cpool.tile([128, 128], bf)
            nc.sync.dma_start(w_in_t[:], w_in_e[:])
            w_out_t = cpool.tile([128, 128], bf)
            nc.sync.dma_start(w_out_t[:], w_out_e[:])
            b_t = cpool.tile([128, 1], f32)
            nc.sync.dma_start(b_t[:], b_e[:])
            iota_t = cpool.tile([128, 128], bf)
            nc.sync.dma_start(iota_t[:], iota_e[:])
            rel_t = cpool.tile([128, G], bf)
            nc.scalar.dma_start(rel_t[:], rel_e[:])
            idx_t = cpool.tile([128, E // 16], mybir.dt.int16)
            nc.scalar.dma_start(idx_t[:], idx_e[:])

            # ---------------- Phase A: f table ----------------
            QW = A_PAD // 4
            with (
                tc.tile_pool(name="pha", bufs=4) as apool,
                tc.tile_pool(name="psA", bufs=3, space="PSUM") as psA,
            ):
                f_sb = None
                # hoist all chunk loads: they sit first on the sync ring
                # (FIFO), so no chunk load waits behind an f-write's sem
                xqs = []
                for x4 in range(4):
                    xq_t = apool.tile([128, QW], bf, tag="xq")
                    nc.sync.dma_start(xq_t[:], xT_e[:, x4 * QW:(x4 + 1) * QW])
                    xqs.append(xq_t)
                for x4 in range(4):
                    xq_t = xqs[x4]
                    for t4q in range(QW // 512):
                        t4 = x4 * (QW // 512) + t4q
                        ps = psA.tile([128, 4, 128], f32)
                        for q in range(4):
                            tl = t4q * 4 + q
                            nc.tensor.matmul(
                                ps[:, q, :],
                                xq_t[:, tl * 128:(tl + 1) * 128],
                                w_in_t[:],
                                start=True, stop=True,
                            )
                        j = t4 % 2
                        if j == 0:
                            f_sb = apool.tile([128, 8, F], bf, tag="fsb")
                        # split PSUM->SBUF casts across Vector and Scalar
                        if t4 % 2 == 0:
                            nc.vector.tensor_copy(
                                f_sb[:, j * 4:(j + 1) * 4, :], ps[:])
                        else:
                            nc.scalar.copy(
                                f_sb[:, j * 4:(j + 1) * 4, :], ps[:])
                        if j == 1:
                            m = t4 // 2          # j-rows [8m, 8m+8)
                            nc.sync.dma_start(
                                f_hbm[:, 8 * m:8 * m + 8, :], f_sb[:])

            # ---------------- Phase B: edges ----------------
            with (
                tc.tile_pool(name="phb", bufs=3) as bpool,
                tc.tile_pool(name="fjp", bufs=6) as fjpool,
                tc.tile_pool(name="psC", bufs=2, space="PSUM") as pscp,
                tc.tile_pool(name="ps2", bufs=2, space="PSUM") as ps2p,
            ):
                psc = None
                for wk in range(WIN_PER_CORE):
                    ch = wk // 4
                    col = WIN * (wk % 4)

                    # Delay the first windows' streaming loads so the
                    # phase-A table chain gets the DMA bandwidth first.
                    delay = 0.012 if wk < 3 else 0
                    with tc.tile_wait_until(delay, enable=wk < 3):
                        w_t = bpool.tile([128, k, F], bf, tag="w")
                        nc.scalar.dma_start(
                            w_t[:], w_ed_e[:, wk * k:(wk + 1) * k, :])
                    # S one-hot generated on DVE from rel values:
                    # S[e, g, a] = (rel[e, g] == a)
                    s_t = bpool.tile([128, k, WIN], mybir.dt.float8e4,
                                     tag="s")
                    nc.vector.tensor_tensor(
                        s_t[:],
                        rel_t[:, wk * k:(wk + 1) * k]
                        .unsqueeze(-1).broadcast_to([128, k, WIN]),
                        iota_t[:].unsqueeze(1).broadcast_to([128, k, WIN]),
                        mybir.AluOpType.is_equal)
                    base8 = wk * k * 8
                    fj_t = fjpool.tile([128, k, F], bf, tag="fj")
                    # 4 gather calls per window, one per SWDGE queue
                    kq = k // 4
                    for piece in range(4):
                        g0, g1 = piece * kq, (piece + 1) * kq
                        if piece == 3:
                            g1 = k
                        nc.gpsimd.dma_gather(
                            fj_t[:, g0:g1, :],
                            f_hbm[:].rearrange("p j f -> (p j) f"),
                            idx_t[:, base8 + g0 * 8:base8 + g1 * 8],
                            num_idxs=(g1 - g0) * 128,
                            num_idxs_reg=(g1 - g0) * 128,
                            elem_size=F,
                            single_packet=False,
                            queue_num=(piece + wk) % 4,
                        )

                    wf_t = bpool.tile([128, k, F], bf, tag="wf")
                    nc.vector.tensor_tensor(
                        wf_t[:], w_t[:], fj_t[:], mybir.AluOpType.mult)

                    if wk % 4 == 0:
                        psc = pscp.tile([128, CHUNK], f32)
                    for g in range(k):
                        nc.tensor.matmul(
                            psc[:, col:col + WIN],
                            wf_t[:, g, :],
                            s_t[:, g, :],
                            start=(g == 0), stop=(g == k - 1),
                        )

                    if wk % 4 == 3:
                        convT = bpool.tile([128, CHUNK], bf, tag="convT")
                        nc.vector.tensor_copy(convT[:], psc[:])
                        ps2 = ps2p.tile([128, CHUNK], f32)
                        nc.tensor.matmul(ps2[:], w_out_t[:], convT[:],
                                         start=True, stop=True)
                        outT = bpool.tile([128, CHUNK], f32, tag="outT")
                        nc.scalar.activation(
                            outT[:], ps2[:],
                            mybir.ActivationFunctionType.Identity,
                            bias=b_t[:],
                        )
                        nc.sync.dma_start(
                            out_e[:, ch * CHUNK:(ch + 1) * CHUNK], outT[:])

    nc.compile()
    _BUILD_CACHE[k] = nc
    return nc


def _prep(x, w_ij, seg_i, idx_j, W_in, W_out, b_out):
    """Host-side sharding: reorder/pad edges, build S one-hots, wrap idxs."""
    x = np.asarray(x, dtype=np.float32)
    w_ij = np.asarray(w_ij, dtype=np.float32)
    seg = np.asarray(seg_i).astype(np.int64)
    idxj = np.asarray(idx_j).astype(np.int64)

    # Relabel atoms so every 128-atom window gets a near-equal edge count
    # (snake-deal atoms in decreasing edge-count order over the windows).
    cnt = np.bincount(seg, minlength=N_ATOMS)
    order = np.argsort(-cnt, kind="stable")
    i = np.arange(N_ATOMS)
    r, c = np.divmod(i, N_WIN)
    w = np.where(r % 2 == 0, c, N_WIN - 1 - c)
    perm = np.empty(N_ATOMS, np.int64)
    perm[order] = w * WIN + r
    seg = perm[seg]
    idxj = perm[idxj]
    o = np.argsort(seg, kind="stable")
    seg, idxj, w_ij = seg[o], idxj[o], w_ij[o]

    bounds = np.searchsorted(seg, np.arange(N_WIN + 1) * WIN)
    n_win = np.diff(bounds)
    k = max(1, int(np.ceil(n_win.max() / 128)))
    e_win = k * 128
    g_core = WIN_PER_CORE * k
    e_pad = g_core * 128

    # Gather-table row for atom a: (p, j) = (a % 128, a // 128);
    # partition-major row = p*JROWS + j.
    grow = ((idxj % 128) * JROWS + idxj // 128).astype(np.int16)

    # padded edge-id + gather-idx matrices
    eidx = np.zeros((N_WIN, e_win), np.int64)
    valid = np.zeros((N_WIN, e_win), bool)
    gidx = np.zeros((N_WIN, e_win), np.int16)
    for kw in range(N_WIN):
        b0, b1 = bounds[kw], bounds[kw + 1]
        n = b1 - b0
        eidx[kw, :n] = np.arange(b0, b1)
        valid[kw, :n] = True
        gidx[kw, :n] = grow[b0:b1]

    w_bf = w_ij.astype(BF16)

    xT = np.zeros((128, A_PAD), BF16)
    xT[:, perm] = np.ascontiguousarray(x.T).astype(BF16)
    shared = {
        "xT": xT,
        "w_in": np.asarray(W_in, np.float32).astype(BF16),
        "w_out": np.asarray(W_out, np.float32).astype(BF16),
        "b_out": np.asarray(b_out, np.float32).reshape(128, 1).copy(),
        "iota": np.tile(np.arange(128, dtype=np.float32).astype(BF16),
                        (128, 1)),
    }

    in_maps = []
    for c in range(N_CORES):
        sl = slice(c * WIN_PER_CORE, (c + 1) * WIN_PER_CORE)
        ei = eidx[sl].reshape(-1)
        va = valid[sl].reshape(-1)

        w_rows = np.zeros((e_pad, F), BF16)
        w_rows[va] = w_bf[ei[va]]
        w_ed = np.ascontiguousarray(
            w_rows.reshape(g_core, 128, F).transpose(1, 0, 2))

        wb = (np.arange(c * WIN_PER_CORE, (c + 1) * WIN_PER_CORE)
              * WIN).repeat(e_win)
        rel = np.where(va, seg[ei] - wb, 0)
        # rel value per (e-partition, group), bf16 (0..127 exact)
        rel_ed = np.ascontiguousarray(
            rel.reshape(g_core, 128).T.astype(np.float32)).astype(BF16)

        # wrapped idx layout: per window, contiguous [16, k*8] wraps
        gi = gidx[sl]                              # [20, e_win]
        blocks = [gi[wkk].reshape(-1, 16).T for wkk in range(WIN_PER_CORE)]
        idxw = np.ascontiguousarray(
            np.tile(np.concatenate(blocks, axis=1), (8, 1)))

        m = dict(shared)
        m["w_ed"] = w_ed
        m["rel_ed"] = rel_ed
        m["idxw"] = idxw
        in_maps.append(m)
    return k, in_maps, perm


def kernel(x, w_ij, seg_i, idx_j, seg_i_sum, W_in, W_out, b_out):
    k, in_maps, perm = _prep(x, w_ij, seg_i, idx_j, W_in, W_out, b_out)
    nc = _build(k)
    res = run_bass_kernel_spmd(nc, in_maps, core_ids=list(range(N_CORES)),
                               trace=TRACE)
    kernel.last_result = res
    # out^T per core: [128 fo, 2560 atoms] -> [2560, 128]
    out = np.concatenate(
        [np.asarray(res.results[c]["out"]).T for c in range(N_CORES)], axis=0)
    return np.ascontiguousarray(out[perm]).astype(np.float32)
